# revision 24
# baseline (speedup 1.0000x reference)
"""Trainium2 Bass kernel: 2-layer adjacency-gated multi-head attention encoder.

Strategy: data-parallel over batch (B=8 -> one batch element per NeuronCore,
8 cores). Weights and the adjacency mask are replicated; no collectives.

v2 changes over v1 (414us baseline):
  - score matmuls in f32r, emitted as head PAIRS on PE row-groups (0,0)/(64,0)
    so the two K=64 matmuls run concurrently in the array (2x) and present
    full-array activity to the HAM clock gate.
  - adj stored bf16 (half the DMA + SBUF).
  - exp batched per (head, sh): one ACT op over [128, 4096] instead of 4.
  - reciprocal runs directly on the PSUM denominator row (no copy).
  - LN applies (x-mu)*rstd via one ACT affine (scale/bias ports); gamma/beta
    on the idle GpSimd engine; stats batched per sh-half so layer-boundary
    transposes/projections pipeline per half.
  - transpose evac via DMA instead of ACT copies; weight DMAs sliced per
    (matrix, column-chunk) so the first qk matmuls start ~5us in.
"""

import math
import os

import ml_dtypes
import numpy as np

import concourse.bass as bass
import concourse.bacc as bacc
import concourse.mybir as mybir
import concourse.tile as tile
from concourse import library_config
from concourse.bass_utils import run_bass_kernel_spmd
from concourse.masks import make_identity

P = 128
S = 1024
E = 512
H = 8
D = 64
L = 2
NE = E // P  # 4 e-chunks
NS = S // P  # 8 s-chunks
NSH = 2      # s halves of 512 (psum free dim)
HC = H // 2  # 4 head pairs
FREE = 512
LN_EPS = 1e-5

F32 = mybir.dt.float32
BF16 = mybir.dt.bfloat16
MM_DT = mybir.dt.float32r
AF = mybir.ActivationFunctionType
OP = mybir.AluOpType


def build_nc():
    debug = os.environ.get("KERNEL_DEBUG", "0") == "1"
    nc = bacc.Bacc(None, target_bir_lowering=False)

    xT_d = nc.declare_dram_parameter("xT", [E, S], MM_DT, isOutput=False)
    xn_d = nc.declare_dram_parameter("xn", [S, E], F32, isOutput=False)
    wts_d = nc.declare_dram_parameter("wts", [L, 4, E, E], MM_DT, isOutput=False)
    pb_d = nc.declare_dram_parameter("pb", [L, 2, P, NE], F32, isOutput=False)
    fb_d = nc.declare_dram_parameter("fb", [L, 2, P, E], F32, isOutput=False)
    adjT_d = nc.declare_dram_parameter("adjT", [S, S], BF16, isOutput=False)
    out_d = nc.declare_dram_parameter("out", [S, E], F32, isOutput=True)
    if debug:
        dbg_q_d = nc.declare_dram_parameter("dbg_q", [P, NE, S], F32, isOutput=True)
        dbg_k_d = nc.declare_dram_parameter("dbg_k", [P, NE, S], F32, isOutput=True)
        dbg_e_d = nc.declare_dram_parameter(
            "dbg_e", [2, P, NS, FREE], BF16, isOutput=True
        )
        dbg_at_d = nc.declare_dram_parameter("dbg_at", [P, NE, S], F32, isOutput=True)
        dbg_xn_d = nc.declare_dram_parameter("dbg_xn", [P, NS, E], F32, isOutput=True)

    with tile.TileContext(nc) as tc:
        with (
            tc.tile_pool(name="const", bufs=1) as const_p,
            tc.tile_pool(name="adj", bufs=NS) as adj_p,
            tc.tile_pool(name="xt", bufs=1) as xt_p,
            tc.tile_pool(name="xn", bufs=1) as xn_p,
            tc.tile_pool(name="w", bufs=1) as w_p,
            tc.tile_pool(name="qk", bufs=1) as qk_p,
            tc.tile_pool(name="v", bufs=NS) as v_p,
            tc.tile_pool(name="exp", bufs=2) as exp_p,
            tc.tile_pool(name="at", bufs=1) as at_p,
            tc.tile_pool(name="small", bufs=2) as small_p,
            tc.tile_pool(name="scr", bufs=2) as scr_p,
            tc.tile_pool(name="ps_s", bufs=2, space="PSUM") as ps_s,
            tc.tile_pool(name="ps_o", bufs=2, space="PSUM") as ps_o,
        ):
            ident = const_p.tile([P, P], F32, tag="ident")
            make_identity(nc, ident)
            eps_t = const_p.tile([P, 1], F32, tag="eps")
            nc.vector.memset(eps_t[:], float(LN_EPS))
            nc.gpsimd.load_library(library_config.attn)

            # ---- initial loads: qk-critical tensors first ----
            xT0 = xt_p.tile([P, NE, S], MM_DT, tag="xt")
            for sh in range(NSH):
                nc.sync.dma_start(
                    out=xT0[:, :, sh * FREE : (sh + 1) * FREE],
                    in_=xT_d[:, sh * FREE : (sh + 1) * FREE].rearrange(
                        "(c p) s -> p c s", p=P
                    ),
                )

            w_t = None
            xn_t = xn_p.tile([P, NS, E], F32, tag="xn")
            adj_t = []

            def load_w(layer, ms):
                nonlocal w_t
                if layer == 0 and w_t is None:
                    w_t = [
                        w_p.tile([P, NE, E], MM_DT, tag=f"w{m}", name=f"w{m}")
                        for m in range(4)
                    ]
                for m in ms:
                    for fc in range(NE):
                        nc.sync.dma_start(
                            out=w_t[m][:, :, fc * P : (fc + 1) * P],
                            in_=wts_d[layer, m].rearrange("(c p) f -> p c f", p=P)[
                                :, :, fc * P : (fc + 1) * P
                            ],
                        )

            xT_cur = xT0
            for layer in range(L):
                # q/k weights + per-partition bias columns first
                load_w(layer, (0, 1))
                pb_t = small_p.tile([P, 2, NE], F32, tag="pb")
                nc.sync.dma_start(
                    out=pb_t[:], in_=pb_d[layer].rearrange("b p c -> p b c")
                )

                # ---- q/k projections -> qT/kT [f, s] (f on partitions) ----
                qkT = []
                for m in range(2):
                    dst = qk_p.tile([P, NE, S], MM_DT, tag=f"qk{m}")
                    for fc in range(NE):
                        for sh in range(NSH):
                            ps2 = ps_s.tile([P, 2, FREE], F32, tag="ps2")
                            ps = ps2[:, 0, :]
                            for ec in range(NE):
                                nc.tensor.matmul(
                                    ps,
                                    w_t[m][:, ec, fc * P : (fc + 1) * P],
                                    xT_cur[:, ec, sh * FREE : (sh + 1) * FREE],
                                    start=(ec == 0),
                                    stop=(ec == NE - 1),
                                )
                            nc.scalar.activation(
                                dst[:, fc, sh * FREE : (sh + 1) * FREE],
                                ps,
                                AF.Identity,
                                bias=pb_t[:, m, fc : fc + 1],
                            )
                    qkT.append(dst)
                qT, kT = qkT
                if debug and layer == 0:
                    nc.sync.dma_start(out=dbg_q_d[:], in_=qT[:].bitcast(F32))
                    nc.sync.dma_start(out=dbg_k_d[:], in_=kT[:].bitcast(F32))

                # ---- remaining loads for this layer (v/o weights, adj, xn) ----
                load_w(layer, (2,))
                if layer == 0:
                    for kc in range(NS):
                        a = adj_p.tile([P, S], BF16, tag="adj")
                        nc.sync.dma_start(
                            out=a[:], in_=adjT_d[kc * P : (kc + 1) * P, :]
                        )
                        adj_t.append(a)
                    nc.sync.dma_start(
                        out=xn_t[:], in_=xn_d[:].rearrange("(c p) e -> p c e", p=P)
                    )
                load_w(layer, (3,))
                fb_g = small_p.tile([P, E], F32, tag="fb_g")
                nc.sync.dma_start(out=fb_g[:], in_=fb_d[layer, 0])
                fb_b = small_p.tile([P, E], F32, tag="fb_b")
                nc.sync.dma_start(out=fb_b[:], in_=fb_d[layer, 1])

                # ---- v projection -> [s, (h, d|1|0pad)] zero-padded to 128
                # cols so attnV matmuls present a full-array shape ----
                v_t = []
                for sc in range(NS):
                    vt = v_p.tile([P, H, P], BF16, tag="v")
                    nc.gpsimd.memset(vt[:, :, D + 1 :], 0.0)
                    nc.gpsimd.memset(vt[:, :, D : D + 1], 1.0)
                    ps2 = ps_s.tile([P, 2, FREE], F32, tag="ps2")
                    ps = ps2[:, 0, :]
                    for ec in range(NE):
                        nc.tensor.matmul(
                            ps,
                            xT_cur[:, ec, sc * P : (sc + 1) * P],
                            w_t[2][:, ec, :],
                            start=(ec == 0),
                            stop=(ec == NE - 1),
                        )
                    nc.any.tensor_copy(
                        vt[:, :, 0:D], ps.rearrange("p (h d) -> p h d", d=D)
                    )
                    v_t.append(vt)

                # ---- attention: head pairs on PE row groups ----
                at_t = at_p.tile([P, NE, S], MM_DT, tag="at")
                for hc in range(HC):
                    for sh in range(NSH):
                        sq = slice(sh * FREE, (sh + 1) * FREE)
                        e2 = exp_p.tile(
                            [P, 2, NS, FREE], BF16, tag="exp", name="e2"
                        )
                        for kc in range(NS):
                            kblk = slice(kc * P, (kc + 1) * P)
                            ps2 = ps_s.tile([P, 2, FREE], F32, tag="ps2")
                            for hh in range(2):
                                hr = slice(hh * D, (hh + 1) * D)
                                nc.tensor.matmul(
                                    ps2[:, hh, :],
                                    kT[hr, hc, kblk],
                                    qT[hr, hc, sq],
                                    start=True,
                                    stop=True,
                                )
                            for hh in range(2):
                                nc.vector.tensor_tensor(
                                    e2[:, hh, kc, :],
                                    ps2[:, hh, :],
                                    adj_t[kc][:, sq],
                                    OP.mult,
                                )
                        for hh in range(2):
                            nc.scalar.activation(e2[:, hh], e2[:, hh], AF.Exp)
                        if debug and layer == 0 and hc == 0 and sh == 0:
                            for hh in range(2):
                                nc.sync.dma_start(out=dbg_e_d[hh], in_=e2[:, hh])
                        po2 = ps_o.tile([P, 2, FREE], F32, tag="po2")
                        for hh in range(2):
                            h = 2 * hc + hh
                            for kc in range(NS):
                                nc.tensor.matmul(
                                    po2[:, hh, :],
                                    v_t[kc][:, h, :],
                                    e2[:, hh, kc, :],
                                    start=(kc == 0),
                                    stop=(kc == NS - 1),
                                )
                        # denominator row for both heads -> SBUF (the custom
                        # DVE reciprocal reads SBUF only), recip + broadcast
                        drow2 = scr_p.tile([1, 2, FREE], F32, tag="drow2", bufs=1)
                        nc.scalar.copy(drow2[:], po2[D : D + 1, :, :])
                        rrow2 = scr_p.tile([1, 2, FREE], F32, tag="rrow2", bufs=1)
                        nc.vector.reciprocal_approx_fast(rrow2[:], drow2[:])
                        rbc2 = scr_p.tile([D, 2, FREE], F32, tag="rbc2", bufs=1)
                        nc.gpsimd.partition_broadcast(rbc2[:], rrow2[:])
                        for hh in range(2):
                            nc.vector.scalar_tensor_tensor(
                                at_t[hh * D : (hh + 1) * D, hc, sq],
                                po2[0:D, hh, :],
                                1.0,
                                rbc2[:, hh, :],
                                OP.mult,
                                OP.mult,
                            )

                if debug and layer == 0:
                    nc.sync.dma_start(out=dbg_at_d[:], in_=at_t[:].bitcast(F32))

                # ---- out projection + residual + LayerNorm (per sh half) ----
                ssum = small_p.tile([P, NS], F32, tag="ssum")
                ssq = small_p.tile([P, NS], F32, tag="ssq")
                negmu = small_p.tile([P, NS], F32, tag="negmu")
                musq = small_p.tile([P, NS], F32, tag="musq")
                sd = small_p.tile([P, NS], F32, tag="sd")
                rstd = small_p.tile([P, NS], F32, tag="rstd")
                nmr = small_p.tile([P, NS], F32, tag="nmr")
                if layer < L - 1:
                    xT_next = xt_p.tile([P, NE, S], MM_DT, tag="xt")
                for sh in range(NSH):
                    cols = slice(sh * 4, sh * 4 + 4)
                    for sc in range(sh * 4, sh * 4 + 4):
                        ps2 = ps_s.tile([P, 2, FREE], F32, tag="ps2")
                        ps = ps2[:, 0, :]
                        for ec in range(NE):
                            nc.tensor.matmul(
                                ps,
                                at_t[:, ec, sc * P : (sc + 1) * P],
                                w_t[3][:, ec, :],
                                start=(ec == 0),
                                stop=(ec == NE - 1),
                            )
                        nc.vector.scalar_tensor_tensor(
                            xn_t[:, sc, :],
                            ps,
                            1.0,
                            xn_t[:, sc, :],
                            OP.mult,
                            OP.add,
                            accum_out=ssum[:, sc : sc + 1],
                        )
                        sq_scr = scr_p.tile([P, E], F32, tag="sqscr", bufs=1)
                        nc.scalar.activation(
                            sq_scr[:],
                            xn_t[:, sc, :],
                            AF.Square,
                            accum_out=ssq[:, sc : sc + 1],
                        )
                    # batched LN stats for this half
                    nc.vector.tensor_scalar_mul(
                        negmu[:, cols], ssum[:, cols], -1.0 / E
                    )
                    nc.vector.tensor_tensor(
                        musq[:, cols], negmu[:, cols], negmu[:, cols], OP.mult
                    )
                    nc.vector.scalar_tensor_tensor(
                        sd[:, cols],
                        ssq[:, cols],
                        1.0 / E,
                        musq[:, cols],
                        OP.mult,
                        OP.subtract,
                    )
                    nc.scalar.activation(sd[:, cols], sd[:, cols], AF.Sqrt, bias=eps_t[:])
                    nc.vector.reciprocal_approx_fast(rstd[:, cols], sd[:, cols])
                    nc.vector.tensor_tensor(
                        nmr[:, cols], negmu[:, cols], rstd[:, cols], OP.mult
                    )
                    for sc in range(sh * 4, sh * 4 + 4):
                        xsc = xn_t[:, sc, :]
                        # (r - mu) * rstd in one ACT affine pass
                        nc.scalar.activation(
                            xsc,
                            xsc,
                            AF.Identity,
                            bias=nmr[:, sc : sc + 1],
                            scale=rstd[:, sc : sc + 1],
                        )
                        # gamma / beta on the (idle) GpSimd engine
                        nc.gpsimd.tensor_tensor(xsc, xsc, fb_g[:], OP.mult)
                        nc.gpsimd.tensor_tensor(xsc, xsc, fb_b[:], OP.add)
                        if layer == L - 1:
                            nc.sync.dma_start(
                                out=out_d[:].rearrange("(c p) e -> p c e", p=P)[
                                    :, sc, :
                                ],
                                in_=xn_t[:, sc, :],
                            )
                        else:
                            for ec in range(NE):
                                pt2 = ps_s.tile([P, 2, FREE], F32, tag="ps2")
                                pt = pt2[:, 0, 0:P]
                                nc.tensor.transpose(
                                    pt,
                                    xn_t[:, sc, ec * P : (ec + 1) * P],
                                    ident[:],
                                )
                                nc.scalar.copy(
                                    xT_next[:, ec, sc * P : (sc + 1) * P], pt
                                )
                if debug and layer == 0:
                    nc.sync.dma_start(out=dbg_xn_d[:], in_=xn_t[:])
                if layer < L - 1:
                    xT_cur = xT_next

    nc.compile()
    return nc


_NC = None
LAST_RESULT = None


def _get_nc():
    global _NC
    if _NC is None:
        _NC = build_nc()
    return _NC


def prep_inputs(x, adj, Wq, bq, Wk, bk, Wv, bv, Wo, bo, gamma, beta):
    """Host-side layout prep. Returns per-core input maps."""
    f32 = np.float32
    x = np.asarray(x, f32)
    adj = np.asarray(adj, f32)
    Wq = np.asarray(Wq, f32)
    bq = np.asarray(bq, f32)
    Wk = np.asarray(Wk, f32)
    bk = np.asarray(bk, f32)
    Wv = np.asarray(Wv, f32)
    bv = np.asarray(bv, f32)
    Wo = np.asarray(Wo, f32)
    bo = np.asarray(bo, f32)
    gamma = np.asarray(gamma, f32)
    beta = np.asarray(beta, f32)

    inv = f32(1.0 / math.sqrt(D))
    # einsum('bse,fe->bsf') => out = x @ W.T, contraction over e. lhsT layout
    # wants W.T = [e, f]. Scale folded into Wq/bq.
    wts = np.stack(
        [
            (Wq * inv).transpose(0, 2, 1),
            Wk.transpose(0, 2, 1),
            Wv.transpose(0, 2, 1),
            Wo.transpose(0, 2, 1),
        ],
        axis=1,
    ).astype(f32)  # [L, 4, e, f]
    wts = np.ascontiguousarray(wts)

    # per-partition bias columns for qT/kT evac: [L, 2, 128, chunk]
    pb = np.stack(
        [
            (bq * inv).reshape(L, NE, P).transpose(0, 2, 1),
            bk.reshape(L, NE, P).transpose(0, 2, 1),
        ],
        axis=1,
    ).astype(f32)
    pb = np.ascontiguousarray(pb)

    # fold bv into bo (attn softmax-averages the constant bv straight
    # through: (attn@(v+bv)) @ Wo^T = attn@v @ Wo^T + Wo@bv), then fold
    # next layer's bo_eff into this layer's beta; layer0 bo into initial xn
    bo_eff = bo + np.einsum("lfe,le->lf", Wo, bv)
    beta_eff = beta.copy()
    beta_eff[: L - 1] += bo_eff[1:]
    fb = np.stack(
        [
            np.broadcast_to(gamma[:, None, :], (L, P, E)),
            np.broadcast_to(beta_eff[:, None, :], (L, P, E)),
        ],
        axis=1,
    ).astype(f32)
    fb = np.ascontiguousarray(fb)

    adjT = np.ascontiguousarray(adj.T.astype(ml_dtypes.bfloat16))

    in_maps = []
    for b in range(x.shape[0]):
        in_maps.append(
            {
                "xT": np.ascontiguousarray(x[b].T),
                "xn": np.ascontiguousarray(x[b] + bo_eff[0][None, :]),
                "wts": wts,
                "pb": pb,
                "fb": fb,
                "adjT": adjT,
            }
        )
    return in_maps


def kernel(x, adj, Wq, bq, Wk, bk, Wv, bv, Wo, bo, gamma, beta):
    global LAST_RESULT
    nc = _get_nc()
    in_maps = prep_inputs(x, adj, Wq, bq, Wk, bk, Wv, bv, Wo, bo, gamma, beta)
    n = len(in_maps)
    trace = os.environ.get("KERNEL_TRACE", "0") == "1"
    res = run_bass_kernel_spmd(nc, in_maps, list(range(n)), trace=trace)
    LAST_RESULT = res
    out = np.stack([res.results[b]["out"] for b in range(n)]).astype(np.float32)
    return out


# revision 26
# speedup vs baseline: 1.1823x; 1.1823x over previous
"""Trainium2 Bass kernel: 2-layer adjacency-gated multi-head attention encoder.

Strategy: data-parallel over batch (B=8 -> one batch element per NeuronCore,
8 cores). Weights and the adjacency mask are replicated; no collectives.

v2 changes over v1 (414us baseline):
  - score matmuls in f32r, emitted as head PAIRS on PE row-groups (0,0)/(64,0)
    so the two K=64 matmuls run concurrently in the array (2x) and present
    full-array activity to the HAM clock gate.
  - adj stored bf16 (half the DMA + SBUF).
  - exp batched per (head, sh): one ACT op over [128, 4096] instead of 4.
  - reciprocal runs directly on the PSUM denominator row (no copy).
  - LN applies (x-mu)*rstd via one ACT affine (scale/bias ports); gamma/beta
    on the idle GpSimd engine; stats batched per sh-half so layer-boundary
    transposes/projections pipeline per half.
  - transpose evac via DMA instead of ACT copies; weight DMAs sliced per
    (matrix, column-chunk) so the first qk matmuls start ~5us in.
"""

import math
import os

import ml_dtypes
import numpy as np

import concourse.bass as bass
import concourse.bacc as bacc
import concourse.mybir as mybir
import concourse.tile as tile
from concourse import library_config
from concourse.bass_utils import run_bass_kernel_spmd
from concourse.masks import make_identity

P = 128
S = 1024
E = 512
H = 8
D = 64
L = 2
NE = E // P  # 4 e-chunks
NS = S // P  # 8 s-chunks
NSH = 2      # s halves of 512 (psum free dim)
HC = H // 2  # 4 head pairs
FREE = 512
LN_EPS = 1e-5

F32 = mybir.dt.float32
BF16 = mybir.dt.bfloat16
MM_DT = mybir.dt.float32r
AF = mybir.ActivationFunctionType
OP = mybir.AluOpType


def build_nc():
    debug = os.environ.get("KERNEL_DEBUG", "0") == "1"
    nc = bacc.Bacc(None, target_bir_lowering=False)

    xT_d = nc.declare_dram_parameter("xT", [E, S], MM_DT, isOutput=False)
    xn_d = nc.declare_dram_parameter("xn", [S, E], F32, isOutput=False)
    wts_d = nc.declare_dram_parameter("wts", [L, 4, E, E], MM_DT, isOutput=False)
    pb_d = nc.declare_dram_parameter("pb", [L, 2, P, NE], F32, isOutput=False)
    fb_d = nc.declare_dram_parameter("fb", [L, 2, P, E], F32, isOutput=False)
    adjT_d = nc.declare_dram_parameter("adjT", [S, S], BF16, isOutput=False)
    out_d = nc.declare_dram_parameter("out", [S, E], F32, isOutput=True)
    if debug:
        dbg_q_d = nc.declare_dram_parameter("dbg_q", [P, NE, S], F32, isOutput=True)
        dbg_k_d = nc.declare_dram_parameter("dbg_k", [P, NE, S], F32, isOutput=True)
        dbg_e_d = nc.declare_dram_parameter(
            "dbg_e", [2, P, NS, FREE], BF16, isOutput=True
        )
        dbg_at_d = nc.declare_dram_parameter("dbg_at", [P, NE, S], F32, isOutput=True)
        dbg_xn_d = nc.declare_dram_parameter("dbg_xn", [P, NS, E], F32, isOutput=True)

    with tile.TileContext(nc) as tc:
        with (
            tc.tile_pool(name="const", bufs=1) as const_p,
            tc.tile_pool(name="adj", bufs=NS) as adj_p,
            tc.tile_pool(name="xt", bufs=1) as xt_p,
            tc.tile_pool(name="xn", bufs=1) as xn_p,
            tc.tile_pool(name="w", bufs=1) as w_p,
            tc.tile_pool(name="qk", bufs=1) as qk_p,
            tc.tile_pool(name="v", bufs=NS) as v_p,
            tc.tile_pool(name="exp", bufs=2) as exp_p,
            tc.tile_pool(name="at", bufs=1) as at_p,
            tc.tile_pool(name="small", bufs=2) as small_p,
            tc.tile_pool(name="scr", bufs=2) as scr_p,
            tc.tile_pool(name="ps_s", bufs=2, space="PSUM") as ps_s,
            tc.tile_pool(name="ps_o", bufs=2, space="PSUM") as ps_o,
        ):
            ident = const_p.tile([P, P], F32, tag="ident")
            make_identity(nc, ident)
            eps_t = const_p.tile([P, 1], F32, tag="eps")
            nc.vector.memset(eps_t[:], float(LN_EPS))
            nc.gpsimd.load_library(library_config.attn)

            # ---- initial loads: qk-critical tensors first ----
            xT0 = xt_p.tile([P, NE, S], MM_DT, tag="xt")
            for sh in range(NSH):
                nc.sync.dma_start(
                    out=xT0[:, :, sh * FREE : (sh + 1) * FREE],
                    in_=xT_d[:, sh * FREE : (sh + 1) * FREE].rearrange(
                        "(c p) s -> p c s", p=P
                    ),
                )

            w_t = None
            xn_t = xn_p.tile([P, NS, E], F32, tag="xn")
            adj_t = []

            def load_w(layer, ms):
                nonlocal w_t
                if layer == 0 and w_t is None:
                    w_t = [
                        w_p.tile([P, NE, E], MM_DT, tag=f"w{m}", name=f"w{m}")
                        for m in range(4)
                    ]
                for m in ms:
                    for fc in range(NE):
                        nc.sync.dma_start(
                            out=w_t[m][:, :, fc * P : (fc + 1) * P],
                            in_=wts_d[layer, m].rearrange("(c p) f -> p c f", p=P)[
                                :, :, fc * P : (fc + 1) * P
                            ],
                        )

            xT_cur = xT0
            for layer in range(L):
                # q/k weights + per-partition bias columns first
                load_w(layer, (0, 1))
                pb_t = small_p.tile([P, 2, NE], F32, tag="pb")
                nc.sync.dma_start(
                    out=pb_t[:], in_=pb_d[layer].rearrange("b p c -> p b c")
                )

                # ---- q/k projections -> qT/kT [f, s] (f on partitions) ----
                qkT = []
                for m in range(2):
                    dst = qk_p.tile([P, NE, S], MM_DT, tag=f"qk{m}")
                    for fc in range(NE):
                        for sh in range(NSH):
                            ps2 = ps_s.tile([P, 2, FREE], F32, tag="ps2")
                            ps = ps2[:, 0, :]
                            for ec in range(NE):
                                nc.tensor.matmul(
                                    ps,
                                    w_t[m][:, ec, fc * P : (fc + 1) * P],
                                    xT_cur[:, ec, sh * FREE : (sh + 1) * FREE],
                                    start=(ec == 0),
                                    stop=(ec == NE - 1),
                                )
                            nc.scalar.activation(
                                dst[:, fc, sh * FREE : (sh + 1) * FREE],
                                ps,
                                AF.Identity,
                                bias=pb_t[:, m, fc : fc + 1],
                            )
                    qkT.append(dst)
                qT, kT = qkT
                if debug and layer == 0:
                    nc.sync.dma_start(out=dbg_q_d[:], in_=qT[:].bitcast(F32))
                    nc.sync.dma_start(out=dbg_k_d[:], in_=kT[:].bitcast(F32))

                # ---- remaining loads for this layer (v/o weights, adj, xn) ----
                load_w(layer, (2,))
                if layer == 0:
                    for kc in range(NS):
                        a = adj_p.tile([P, S], BF16, tag="adj")
                        nc.sync.dma_start(
                            out=a[:], in_=adjT_d[kc * P : (kc + 1) * P, :]
                        )
                        adj_t.append(a)
                    nc.sync.dma_start(
                        out=xn_t[:], in_=xn_d[:].rearrange("(c p) e -> p c e", p=P)
                    )
                load_w(layer, (3,))
                fb_g = small_p.tile([P, E], F32, tag="fb_g")
                nc.sync.dma_start(out=fb_g[:], in_=fb_d[layer, 0])
                fb_b = small_p.tile([P, E], F32, tag="fb_b")
                nc.sync.dma_start(out=fb_b[:], in_=fb_d[layer, 1])

                # ---- v projection -> [s, (h, d|1|0pad)] zero-padded to 128
                # cols so attnV matmuls present a full-array shape ----
                v_t = []
                for sc in range(NS):
                    vt = v_p.tile([P, H, D + 1], BF16, tag="v")
                    nc.vector.memset(vt[:, :, D : D + 1], 1.0)
                    ps2 = ps_s.tile([P, 2, FREE], F32, tag="ps2")
                    ps = ps2[:, 0, :]
                    for ec in range(NE):
                        nc.tensor.matmul(
                            ps,
                            xT_cur[:, ec, sc * P : (sc + 1) * P],
                            w_t[2][:, ec, :],
                            start=(ec == 0),
                            stop=(ec == NE - 1),
                        )
                    nc.any.tensor_copy(
                        vt[:, :, 0:D], ps.rearrange("p (h d) -> p h d", d=D)
                    )
                    v_t.append(vt)

                # ---- attention: head pairs on PE row groups ----
                at_t = at_p.tile([P, NE, S], MM_DT, tag="at")
                for hc in range(HC):
                    for sh in range(NSH):
                        sq = slice(sh * FREE, (sh + 1) * FREE)
                        e2 = exp_p.tile(
                            [P, 2, NS, FREE], BF16, tag="exp", name="e2"
                        )
                        for kc in range(NS):
                            kblk = slice(kc * P, (kc + 1) * P)
                            ps2 = ps_s.tile([P, 2, FREE], F32, tag="ps2")
                            for hh in range(2):
                                hr = slice(hh * D, (hh + 1) * D)
                                nc.tensor.matmul(
                                    ps2[:, hh, :],
                                    kT[hr, hc, kblk],
                                    qT[hr, hc, sq],
                                    start=True,
                                    stop=True,
                                )
                            for hh in range(2):
                                nc.vector.tensor_tensor(
                                    e2[:, hh, kc, :],
                                    ps2[:, hh, :],
                                    adj_t[kc][:, sq],
                                    OP.mult,
                                )
                        for hh in range(2):
                            nc.scalar.activation(e2[:, hh], e2[:, hh], AF.Exp)
                        if debug and layer == 0 and hc == 0 and sh == 0:
                            for hh in range(2):
                                nc.sync.dma_start(out=dbg_e_d[hh], in_=e2[:, hh])
                        po2 = ps_o.tile([P, 2, FREE], F32, tag="po2")
                        for hh in range(2):
                            h = 2 * hc + hh
                            for kc in range(NS):
                                nc.tensor.matmul(
                                    po2[0 : D + 1, hh, :],
                                    v_t[kc][:, h, :],
                                    e2[:, hh, kc, :],
                                    start=(kc == 0),
                                    stop=(kc == NS - 1),
                                )
                        # denominator row for both heads -> SBUF (the custom
                        # DVE reciprocal reads SBUF only), recip + broadcast
                        drow2 = scr_p.tile([1, 2, FREE], F32, tag="drow2", bufs=1)
                        nc.scalar.copy(drow2[:], po2[D : D + 1, :, :])
                        rrow2 = scr_p.tile([1, 2, FREE], F32, tag="rrow2", bufs=1)
                        nc.vector.reciprocal_approx_fast(rrow2[:], drow2[:])
                        rbc2 = scr_p.tile([D, 2, FREE], F32, tag="rbc2", bufs=2)
                        nc.gpsimd.partition_broadcast(rbc2[:], rrow2[:])
                        for hh in range(2):
                            nc.vector.scalar_tensor_tensor(
                                at_t[hh * D : (hh + 1) * D, hc, sq],
                                po2[0:D, hh, :],
                                1.0,
                                rbc2[:, hh, :],
                                OP.mult,
                                OP.mult,
                            )

                if debug and layer == 0:
                    nc.sync.dma_start(out=dbg_at_d[:], in_=at_t[:].bitcast(F32))

                # ---- out projection + residual + LayerNorm (per sh half) ----
                ssum = small_p.tile([P, NS], F32, tag="ssum")
                ssq = small_p.tile([P, NS], F32, tag="ssq")
                negmu = small_p.tile([P, NS], F32, tag="negmu")
                musq = small_p.tile([P, NS], F32, tag="musq")
                sd = small_p.tile([P, NS], F32, tag="sd")
                rstd = small_p.tile([P, NS], F32, tag="rstd")
                nmr = small_p.tile([P, NS], F32, tag="nmr")
                if layer < L - 1:
                    xT_next = xt_p.tile([P, NE, S], MM_DT, tag="xt")
                for sh in range(NSH):
                    cols = slice(sh * 4, sh * 4 + 4)
                    for sc in range(sh * 4, sh * 4 + 4):
                        ps2 = ps_s.tile([P, 2, FREE], F32, tag="ps2")
                        ps = ps2[:, 0, :]
                        for ec in range(NE):
                            nc.tensor.matmul(
                                ps,
                                at_t[:, ec, sc * P : (sc + 1) * P],
                                w_t[3][:, ec, :],
                                start=(ec == 0),
                                stop=(ec == NE - 1),
                            )
                        nc.vector.scalar_tensor_tensor(
                            xn_t[:, sc, :],
                            ps,
                            1.0,
                            xn_t[:, sc, :],
                            OP.mult,
                            OP.add,
                            accum_out=ssum[:, sc : sc + 1],
                        )
                        sq_scr = scr_p.tile([P, E], F32, tag="sqscr", bufs=1)
                        nc.scalar.activation(
                            sq_scr[:],
                            xn_t[:, sc, :],
                            AF.Square,
                            accum_out=ssq[:, sc : sc + 1],
                        )
                    # batched LN stats for this half
                    nc.vector.tensor_scalar_mul(
                        negmu[:, cols], ssum[:, cols], -1.0 / E
                    )
                    nc.vector.tensor_tensor(
                        musq[:, cols], negmu[:, cols], negmu[:, cols], OP.mult
                    )
                    nc.vector.scalar_tensor_tensor(
                        sd[:, cols],
                        ssq[:, cols],
                        1.0 / E,
                        musq[:, cols],
                        OP.mult,
                        OP.subtract,
                    )
                    nc.scalar.activation(sd[:, cols], sd[:, cols], AF.Sqrt, bias=eps_t[:])
                    nc.vector.reciprocal_approx_fast(rstd[:, cols], sd[:, cols])
                    nc.vector.tensor_tensor(
                        nmr[:, cols], negmu[:, cols], rstd[:, cols], OP.mult
                    )
                    for sc in range(sh * 4, sh * 4 + 4):
                        xsc = xn_t[:, sc, :]
                        # (r - mu) * rstd in one ACT affine pass
                        nc.scalar.activation(
                            xsc,
                            xsc,
                            AF.Identity,
                            bias=nmr[:, sc : sc + 1],
                            scale=rstd[:, sc : sc + 1],
                        )
                        # gamma / beta on the (idle) GpSimd engine
                        nc.gpsimd.tensor_tensor(xsc, xsc, fb_g[:], OP.mult)
                        nc.gpsimd.tensor_tensor(xsc, xsc, fb_b[:], OP.add)
                        if layer == L - 1:
                            nc.sync.dma_start(
                                out=out_d[:].rearrange("(c p) e -> p c e", p=P)[
                                    :, sc, :
                                ],
                                in_=xn_t[:, sc, :],
                            )
                        else:
                            for ec in range(NE):
                                pt2 = ps_s.tile([P, 2, FREE], F32, tag="ps2")
                                pt = pt2[:, 0, 0:P]
                                nc.tensor.transpose(
                                    pt,
                                    xn_t[:, sc, ec * P : (ec + 1) * P],
                                    ident[:],
                                )
                                nc.scalar.copy(
                                    xT_next[:, ec, sc * P : (sc + 1) * P], pt
                                )
                if debug and layer == 0:
                    nc.sync.dma_start(out=dbg_xn_d[:], in_=xn_t[:])
                if layer < L - 1:
                    xT_cur = xT_next

    nc.compile()
    return nc


_NC = None
LAST_RESULT = None


def _get_nc():
    global _NC
    if _NC is None:
        _NC = build_nc()
    return _NC


def prep_inputs(x, adj, Wq, bq, Wk, bk, Wv, bv, Wo, bo, gamma, beta):
    """Host-side layout prep. Returns per-core input maps."""
    f32 = np.float32
    x = np.asarray(x, f32)
    adj = np.asarray(adj, f32)
    Wq = np.asarray(Wq, f32)
    bq = np.asarray(bq, f32)
    Wk = np.asarray(Wk, f32)
    bk = np.asarray(bk, f32)
    Wv = np.asarray(Wv, f32)
    bv = np.asarray(bv, f32)
    Wo = np.asarray(Wo, f32)
    bo = np.asarray(bo, f32)
    gamma = np.asarray(gamma, f32)
    beta = np.asarray(beta, f32)

    inv = f32(1.0 / math.sqrt(D))
    # einsum('bse,fe->bsf') => out = x @ W.T, contraction over e. lhsT layout
    # wants W.T = [e, f]. Scale folded into Wq/bq.
    wts = np.stack(
        [
            (Wq * inv).transpose(0, 2, 1),
            Wk.transpose(0, 2, 1),
            Wv.transpose(0, 2, 1),
            Wo.transpose(0, 2, 1),
        ],
        axis=1,
    ).astype(f32)  # [L, 4, e, f]
    wts = np.ascontiguousarray(wts)

    # per-partition bias columns for qT/kT evac: [L, 2, 128, chunk]
    pb = np.stack(
        [
            (bq * inv).reshape(L, NE, P).transpose(0, 2, 1),
            bk.reshape(L, NE, P).transpose(0, 2, 1),
        ],
        axis=1,
    ).astype(f32)
    pb = np.ascontiguousarray(pb)

    # fold bv into bo (attn softmax-averages the constant bv straight
    # through: (attn@(v+bv)) @ Wo^T = attn@v @ Wo^T + Wo@bv), then fold
    # next layer's bo_eff into this layer's beta; layer0 bo into initial xn
    bo_eff = bo + np.einsum("lfe,le->lf", Wo, bv)
    beta_eff = beta.copy()
    beta_eff[: L - 1] += bo_eff[1:]
    fb = np.stack(
        [
            np.broadcast_to(gamma[:, None, :], (L, P, E)),
            np.broadcast_to(beta_eff[:, None, :], (L, P, E)),
        ],
        axis=1,
    ).astype(f32)
    fb = np.ascontiguousarray(fb)

    adjT = np.ascontiguousarray(adj.T.astype(ml_dtypes.bfloat16))

    in_maps = []
    for b in range(x.shape[0]):
        in_maps.append(
            {
                "xT": np.ascontiguousarray(x[b].T),
                "xn": np.ascontiguousarray(x[b] + bo_eff[0][None, :]),
                "wts": wts,
                "pb": pb,
                "fb": fb,
                "adjT": adjT,
            }
        )
    return in_maps


def kernel(x, adj, Wq, bq, Wk, bk, Wv, bv, Wo, bo, gamma, beta):
    global LAST_RESULT
    nc = _get_nc()
    in_maps = prep_inputs(x, adj, Wq, bq, Wk, bk, Wv, bv, Wo, bo, gamma, beta)
    n = len(in_maps)
    trace = os.environ.get("KERNEL_TRACE", "0") == "1"
    res = run_bass_kernel_spmd(nc, in_maps, list(range(n)), trace=trace)
    LAST_RESULT = res
    out = np.stack([res.results[b]["out"] for b in range(n)]).astype(np.float32)
    return out
